# revision 4
# baseline (speedup 1.0000x reference)
"""D2Q9 Lattice-Boltzmann single step (collide + stream + bounce-back + lift)
on 8 Trainium2 NeuronCores.

Contract: kernel(**inputs) takes FULL inputs
  f [2048,2048,9] f32, rho [2048,2048] f32, u [2048,2048,2] f32,
  obstacle_mask [2048,2048] bool
and returns the FULL output [2048,2048,12] f32
  (f_new[9], rho_new, u_new[2] packed on the last axis).

Sharding: 1-D domain decomposition over the first spatial axis; each core
gets 256 rows plus a 1-row halo on each side (built host-side with wraparound,
so the device kernel is a purely local stencil). The 1-column y-halo is also
host-padded.
"""

import numpy as np
import concourse.bass as bass
import concourse.bacc as bacc
import concourse.mybir as mybir
from concourse import tile
from concourse.bass_utils import run_bass_kernel_spmd

NX = 2048
NY = 2048
NCORES = 8
R = NX // NCORES          # 256 rows per core
SLAB = R + 2              # 258 rows incl halos
YP = NY + 2               # 2050 cols incl halos

TAU = 0.6
INV_TAU = 1.0 / TAU       # 5/3
FCOEF = 1.0 - INV_TAU     # -2/3   (f coefficient in f* = FCOEF*f + INV_TAU*feq)
W1P = INV_TAU * (1.0 / 9.0)    # 5/27  (axis dirs, scaled by 1/tau)
W5P = INV_TAU * (1.0 / 36.0)   # 5/108 (diag dirs)
W0P = INV_TAU * (4.0 / 9.0)    # 20/27 (rest dir)

EX = [0, 1, 0, -1, 0, 1, -1, -1, 1]
EY = [0, 0, 1, 0, -1, 1, 1, -1, -1]
OPP = [0, 3, 4, 1, 2, 7, 8, 5, 6]

W = 512                   # y-chunk width
NCHUNK = NY // W          # 4
F = W + 2                 # chunk width incl y-halo
FP32 = mybir.dt.float32

# main x-tiles: f_star row ranges [base, base+128); fix-up covers the gap
TILE_BASES = [0, 130]
# fix-up: f_star slab rows 126..131 (6 rows), out slab rows 127..130
FX_FS0 = 126
FX_NFS = 6
FX_OUT0 = 127
FX_NOUT = 4
FX_SEG = 4
FX_W = NY // FX_SEG       # 512
FX_F = FX_W + 2

AL = mybir.AluOpType


def _v3(ap):
    """view a [P, N] AP as [P, N, 1] so all operands rank-match."""
    return ap.rearrange("p (x c) -> p x c", c=1)


def _collide(nc, scr, P, FW, fv, rv, ux, uy, fstar, tagp):
    """Emit the collision stage: fills the 9 fstar planes [P, FW]."""
    vec = nc.vector

    def t(name):
        tl = scr.tile([P, FW], FP32, tag=f"{tagp}{name}")
        return _v3(tl[:])

    r1 = t("r1"); r2 = t("r2"); t1 = t("t1"); t2 = t("t2")
    usqr = t("usqr"); Pv = t("P"); sv = t("s"); dv = t("d")
    rsv = t("rs"); rdv = t("rd"); a5 = t("A5"); a6 = t("A6")
    pw9 = t("pw9"); pw36 = t("pw36"); pw49 = t("pw49")
    g1 = t("G1"); g2 = t("G2"); g5 = t("G5"); g6 = t("G6")
    feqv = []
    for i in range(1, 9):
        fq = t(f"feq{i}")
        feqv.append(fq)

    vec.tensor_tensor(r1, rv, ux, AL.mult)
    vec.tensor_tensor(r2, rv, uy, AL.mult)
    vec.tensor_tensor(t1, ux, r1, AL.mult)
    vec.tensor_tensor(t2, uy, r2, AL.mult)
    vec.tensor_tensor(usqr, t1, t2, AL.add)
    vec.scalar_tensor_tensor(Pv, usqr, -1.5, rv, AL.mult, AL.add)
    vec.tensor_tensor(sv, ux, uy, AL.add)
    vec.tensor_tensor(dv, ux, uy, AL.subtract)
    vec.tensor_tensor(rsv, r1, r2, AL.add)
    vec.tensor_tensor(rdv, r1, r2, AL.subtract)
    vec.tensor_tensor(a5, sv, rsv, AL.mult)
    vec.tensor_tensor(a6, dv, rdv, AL.mult)
    nc.scalar.mul(pw9, Pv, W1P)
    nc.scalar.mul(pw36, Pv, W5P)
    nc.scalar.mul(pw49, Pv, W0P)
    vec.scalar_tensor_tensor(g1, t1, 4.5 * W1P, pw9, AL.mult, AL.add)
    vec.scalar_tensor_tensor(g2, t2, 4.5 * W1P, pw9, AL.mult, AL.add)
    vec.scalar_tensor_tensor(g5, a5, 4.5 * W5P, pw36, AL.mult, AL.add)
    vec.scalar_tensor_tensor(g6, a6, 4.5 * W5P, pw36, AL.mult, AL.add)
    # feq' (pre-scaled by 1/tau); index in feqv is dir-1
    vec.scalar_tensor_tensor(feqv[0], r1, 3 * W1P, g1, AL.mult, AL.add)   # 1
    vec.scalar_tensor_tensor(feqv[2], r1, -3 * W1P, g1, AL.mult, AL.add)  # 3
    vec.scalar_tensor_tensor(feqv[1], r2, 3 * W1P, g2, AL.mult, AL.add)   # 2
    vec.scalar_tensor_tensor(feqv[3], r2, -3 * W1P, g2, AL.mult, AL.add)  # 4
    vec.scalar_tensor_tensor(feqv[4], rsv, 3 * W5P, g5, AL.mult, AL.add)  # 5
    vec.scalar_tensor_tensor(feqv[6], rsv, -3 * W5P, g5, AL.mult, AL.add) # 7
    vec.scalar_tensor_tensor(feqv[5], rdv, -3 * W5P, g6, AL.mult, AL.add) # 6
    vec.scalar_tensor_tensor(feqv[7], rdv, 3 * W5P, g6, AL.mult, AL.add)  # 8
    fsv = [_v3(fs[:]) for fs in fstar]
    vec.scalar_tensor_tensor(fsv[0], fv[:, :, 0:1], FCOEF, pw49, AL.mult, AL.add)
    for i in range(1, 9):
        vec.scalar_tensor_tensor(fsv[i], fv[:, :, i:i + 1], FCOEF,
                                 feqv[i - 1], AL.mult, AL.add)
    return fsv


def _lift(nc, scr, P, OW, ov, tagp):
    """rho/u lift from the 9 selected slots of ov ([P, OW, 12] view)."""
    vec = nc.vector

    def t(name):
        tl = scr.tile([P, OW], FP32, tag=f"{tagp}{name}")
        return _v3(tl[:])

    # reuse collision scratch tags (those tiles are dead by lift time)
    av = t("feq1"); bv = t("feq2"); cv = t("feq3"); ddv = t("feq4")
    m1 = t("feq5"); m2 = t("feq6")
    t01 = t("feq7"); t23 = t("feq8"); t45 = t("G1"); t67 = t("G2")
    t03 = t("G5"); t47 = t("G6"); t07 = t("pw9"); inv = t("pw36")
    s_ = [ov[:, :, i:i + 1] for i in range(9)]
    vec.tensor_tensor(av, s_[1], s_[3], AL.subtract)
    vec.tensor_tensor(bv, s_[2], s_[4], AL.subtract)
    vec.tensor_tensor(cv, s_[5], s_[7], AL.subtract)
    vec.tensor_tensor(ddv, s_[8], s_[6], AL.subtract)
    vec.tensor_tensor(m1, av, cv, AL.add)
    vec.tensor_tensor(m1, m1, ddv, AL.add)
    vec.tensor_tensor(m2, bv, cv, AL.add)
    vec.tensor_tensor(m2, m2, ddv, AL.subtract)
    vec.tensor_tensor(t01, s_[0], s_[1], AL.add)
    vec.tensor_tensor(t23, s_[2], s_[3], AL.add)
    vec.tensor_tensor(t45, s_[4], s_[5], AL.add)
    vec.tensor_tensor(t67, s_[6], s_[7], AL.add)
    vec.tensor_tensor(t03, t01, t23, AL.add)
    vec.tensor_tensor(t47, t45, t67, AL.add)
    vec.tensor_tensor(t07, t03, t47, AL.add)
    vec.tensor_tensor(ov[:, :, 9:10], t07, s_[8], AL.add)
    vec.reciprocal_approx_fast(inv, ov[:, :, 9:10])
    vec.tensor_tensor(ov[:, :, 10:11], m1, inv, AL.mult)
    vec.tensor_tensor(ov[:, :, 11:12], m2, inv, AL.mult)


def _build_program():
    nc = bacc.Bacc(None)

    f_d = nc.declare_dram_parameter("f", [SLAB, YP, 9], FP32, isOutput=False)
    rho_d = nc.declare_dram_parameter("rho", [SLAB, YP], FP32, isOutput=False)
    u_d = nc.declare_dram_parameter("u", [SLAB, YP, 2], FP32, isOutput=False)
    mask_d = nc.declare_dram_parameter("mask", [SLAB, YP], mybir.dt.uint8, isOutput=False)
    out_d = nc.declare_dram_parameter("out", [R, NY, 12], FP32, isOutput=True)

    with tile.TileContext(nc) as tc:
        # ======================= main tiles =======================
        with (
            tc.tile_pool(name="io", bufs=2) as io,
            tc.tile_pool(name="pln", bufs=1) as pln,
            tc.tile_pool(name="scr", bufs=1) as scr,
        ):
            for tb in TILE_BASES:
                for ch in range(NCHUNK):
                    c0 = ch * W
                    fT = io.tile([128, F * 9], FP32, tag="fT")
                    rhoT = io.tile([128, F], FP32, tag="rhoT")
                    uT = io.tile([128, F * 2], FP32, tag="uT")
                    maskT = io.tile([128, F], mybir.dt.uint8, tag="maskT")
                    outT = io.tile([128, W * 12], FP32, tag="outT")
                    nc.sync.dma_start(out=fT[:], in_=f_d[tb:tb + 128, c0:c0 + F, :])
                    nc.sync.dma_start(out=rhoT[:], in_=rho_d[tb:tb + 128, c0:c0 + F])
                    nc.sync.dma_start(out=uT[:], in_=u_d[tb:tb + 128, c0:c0 + F, :])
                    nc.sync.dma_start(out=maskT[:], in_=mask_d[tb:tb + 128, c0:c0 + F])
                    fstar = [pln.tile([128, F], FP32, tag=f"fs{i}", name=f"fs{i}") for i in range(9)]

                    fv = fT[:].rearrange("p (x c) -> p x c", c=9)
                    uv = uT[:].rearrange("p (x c) -> p x c", c=2)
                    fsv = _collide(nc, scr, 128, F, fv, _v3(rhoT[:]),
                                   uv[:, :, 0:1], uv[:, :, 1:2], fstar, "m_")

                    ov = outT[:].rearrange("p (x c) -> p x c", c=12)
                    mk = _v3(maskT[:])[:, 1:1 + W, :]
                    for i in range(9):
                        exi, eyi = EX[i], EY[i]
                        ysl = slice(1 - eyi, 1 - eyi + W)
                        if exi == 0:
                            src = fsv[i][:, ysl, :]
                        else:
                            sp = pln.tile([128, W], FP32, tag=f"S{i}")
                            if exi == 1:
                                nc.sync.dma_start(out=sp[1:128, :],
                                                  in_=fstar[i][0:127, ysl])
                            else:
                                nc.sync.dma_start(out=sp[0:127, :],
                                                  in_=fstar[i][1:128, ysl])
                            src = _v3(sp[:])
                        nc.scalar.copy(ov[:, :, i:i + 1], src)
                    for i in range(1, 9):
                        nc.vector.copy_predicated(ov[:, :, i:i + 1], mk,
                                                  fsv[OPP[i]][:, 1:1 + W, :])

                    _lift(nc, scr, 128, W, ov, "m_")

                    # valid out rows are partitions 1..126 (slab rows tb+1..)
                    nc.sync.dma_start(
                        out=out_d[tb:tb + 126, c0:c0 + W, :],
                        in_=outT[1:127, :])

        # ================= fix-up pass (out slab rows 127..130) =========
        PF = FX_SEG * FX_NFS      # 24 partitions of f_star rows
        PO = FX_SEG * FX_NOUT     # 16 partitions of out rows
        with (
            tc.tile_pool(name="fio", bufs=1) as fio,
            tc.tile_pool(name="fpln", bufs=1) as fpln,
            tc.tile_pool(name="fscr", bufs=1) as fscr,
        ):
            fxf = fio.tile([PF, FX_F * 9], FP32, tag="fxf")
            fxrho = fio.tile([PF, FX_F], FP32, tag="fxrho")
            fxu = fio.tile([PF, FX_F * 2], FP32, tag="fxu")
            fxmask = fio.tile([PO, FX_F], mybir.dt.uint8, tag="fxmask")
            fxout = fio.tile([PO, FX_W * 12], FP32, tag="fxout")
            for sg in range(FX_SEG):
                c0 = sg * FX_W
                nc.sync.dma_start(
                    out=fxf[sg * FX_NFS:(sg + 1) * FX_NFS, :],
                    in_=f_d[FX_FS0:FX_FS0 + FX_NFS, c0:c0 + FX_F, :])
                nc.sync.dma_start(
                    out=fxrho[sg * FX_NFS:(sg + 1) * FX_NFS, :],
                    in_=rho_d[FX_FS0:FX_FS0 + FX_NFS, c0:c0 + FX_F])
                nc.sync.dma_start(
                    out=fxu[sg * FX_NFS:(sg + 1) * FX_NFS, :],
                    in_=u_d[FX_FS0:FX_FS0 + FX_NFS, c0:c0 + FX_F, :])
                nc.sync.dma_start(
                    out=fxmask[sg * FX_NOUT:(sg + 1) * FX_NOUT, :],
                    in_=mask_d[FX_OUT0:FX_OUT0 + FX_NOUT, c0:c0 + FX_F])
            fxstar = [fpln.tile([PF, FX_F], FP32, tag=f"fxs{i}", name=f"fxs{i}") for i in range(9)]

            fv = fxf[:].rearrange("p (x c) -> p x c", c=9)
            uv = fxu[:].rearrange("p (x c) -> p x c", c=2)
            _collide(nc, fscr, PF, FX_F, fv, _v3(fxrho[:]),
                     uv[:, :, 0:1], uv[:, :, 1:2], fxstar, "fx_")

            # stream shift: out q = sg*4+jj is slab row 127+jj; source f_star
            # partition sg*6 + (jj+1-ex), free offset 1-ey
            ov = fxout[:].rearrange("p (x c) -> p x c", c=12)
            fxB = [None] * 9
            for i in range(9):
                exi, eyi = EX[i], EY[i]
                ysl = slice(1 - eyi, 1 - eyi + FX_W)
                sp = fpln.tile([PO, FX_W], FP32, tag=f"fxS{i}")
                for sg in range(FX_SEG):
                    nc.sync.dma_start(
                        out=sp[sg * FX_NOUT:(sg + 1) * FX_NOUT, :],
                        in_=fxstar[i][sg * FX_NFS + 1 - exi:
                                      sg * FX_NFS + 1 - exi + FX_NOUT, ysl])
                nc.scalar.copy(ov[:, :, i:i + 1], _v3(sp[:]))
                spb = fpln.tile([PO, FX_W], FP32, tag=f"fxB{i}")
                for sg in range(FX_SEG):
                    nc.sync.dma_start(
                        out=spb[sg * FX_NOUT:(sg + 1) * FX_NOUT, :],
                        in_=fxstar[i][sg * FX_NFS + 1:sg * FX_NFS + 1 + FX_NOUT,
                                      1:1 + FX_W])
                fxB[i] = spb
            mk = _v3(fxmask[:])[:, 1:1 + FX_W, :]
            for i in range(1, 9):
                nc.vector.copy_predicated(ov[:, :, i:i + 1], mk, _v3(fxB[OPP[i]][:]))

            _lift(nc, fscr, PO, FX_W, ov, "fx_")

            # out slab rows 127..130 = out_d rows 126..129
            for sg in range(FX_SEG):
                nc.sync.dma_start(
                    out=out_d[FX_OUT0 - 1:FX_OUT0 - 1 + FX_NOUT,
                              sg * FX_W:(sg + 1) * FX_W, :],
                    in_=fxout[sg * FX_NOUT:(sg + 1) * FX_NOUT, :])

    nc.finalize()
    return nc


_NC_CACHE = None


def _get_nc():
    global _NC_CACHE
    if _NC_CACHE is None:
        _NC_CACHE = _build_program()
    return _NC_CACHE


def _pad_slab(arr, lo, hi):
    """rows [lo-1, hi+1) with x wraparound, then 1-col y wraparound halo."""
    rows = np.take(arr, np.arange(lo - 1, hi + 1), axis=0, mode="wrap")
    return np.concatenate([rows[:, -1:], rows, rows[:, :1]], axis=1)


def kernel(f, rho, u, obstacle_mask, _trace=False):
    f = np.ascontiguousarray(f, dtype=np.float32)
    rho = np.ascontiguousarray(rho, dtype=np.float32)
    u = np.ascontiguousarray(u, dtype=np.float32)
    maskf = np.asarray(obstacle_mask).astype(np.uint8)

    in_maps = []
    for k in range(NCORES):
        lo, hi = k * R, (k + 1) * R
        in_maps.append({
            "f": np.ascontiguousarray(_pad_slab(f, lo, hi)),
            "rho": np.ascontiguousarray(_pad_slab(rho, lo, hi)),
            "u": np.ascontiguousarray(_pad_slab(u, lo, hi)),
            "mask": np.ascontiguousarray(_pad_slab(maskf, lo, hi)),
        })

    nc = _get_nc()
    res = run_bass_kernel_spmd(nc, in_maps, list(range(NCORES)),
                               trace=bool(_trace))
    out = np.concatenate([res.results[k]["out"] for k in range(NCORES)], axis=0)
    if _trace:
        return out, res
    return out


# revision 19
# speedup vs baseline: 257.7822x; 257.7822x over previous
"""D2Q9 Lattice-Boltzmann single step (collide + stream + bounce-back + lift)
on 8 Trainium2 NeuronCores.

Contract: kernel(**inputs) takes FULL inputs
  f [2048,2048,9] f32, rho [2048,2048] f32, u [2048,2048,2] f32,
  obstacle_mask [2048,2048] bool
and returns the FULL output [2048,2048,12] f32
  (f_new[9], rho_new, u_new[2] packed on the last axis).

Sharding: 1-D domain decomposition over the first spatial axis; each core
gets 256 rows plus a 1-row halo on each side (built host-side with
wraparound, so the device kernel is a purely local stencil). The 1-column
y-halo is also host-padded. Host packs f, rho, u into one 12-channel
tensor so each tile needs just two loads (data + mask) and one store.

Streaming's +-1 row shift is done with a shifted-identity matmul on the
(otherwise idle) TensorEngine into PSUM; the +-1 column shift is a free-dim
AP offset. Bounce-back is copy_predicated on the obstacle mask."""

import numpy as np
import concourse.bass as bass
import concourse.bacc as bacc
import concourse.mybir as mybir
from concourse import tile
from concourse.bass_utils import run_bass_kernel_spmd

NX = 2048
NY = 2048
NCORES = 8
R = NX // NCORES          # 256 rows per core
SLAB = R + 2              # 258 rows incl halos
YP = NY + 2               # 2050 cols incl halos

TAU = 0.6
INV_TAU = 1.0 / TAU       # 5/3
FCOEF = 1.0 - INV_TAU     # -2/3   (f coefficient in f* = FCOEF*f + INV_TAU*feq)
W1P = INV_TAU * (1.0 / 9.0)    # 5/27  (axis dirs, scaled by 1/tau)
W5P = INV_TAU * (1.0 / 36.0)   # 5/108 (diag dirs)
W0P = INV_TAU * (4.0 / 9.0)    # 20/27 (rest dir)

EX = [0, 1, 0, -1, 0, 1, -1, -1, 1]
EY = [0, 0, 1, 0, -1, 1, 1, -1, -1]
OPP = [0, 3, 4, 1, 2, 7, 8, 5, 6]

W = 512                   # y-chunk width
NCHUNK = NY // W          # 4
F = W + 2                 # chunk width incl y-halo
FP32 = mybir.dt.float32
BF16 = mybir.dt.bfloat16
U8 = mybir.dt.uint8

# main x-tiles: f_star row ranges [base, base+128); fix-up covers the gap
TILE_BASES = [0, 130]
# fix-up: f_star slab rows 126..131 (6 rows), out slab rows 127..130
FX_FS0 = 126
FX_NFS = 6
FX_OUT0 = 127
FX_NOUT = 4
FX_SEG = 8
FX_W = NY // FX_SEG
FX_F = FX_W + 2
FX_PO = FX_SEG * FX_NOUT
FX_PF = FX_SEG * FX_NFS
SHM_COLS = 256 + 3 * FX_PO

AL = mybir.AluOpType


def _v3(ap):
    """view a [P, N] AP as [P, N, 1] so all operands rank-match."""
    return ap.rearrange("p (x c) -> p x c", c=1)


def _collide(nc, scr, P, FW, fu12, rhoT, fstar, tagp):
    """Collision stage. fu12: [P, FW, 12] bf16 view (f0..f8, ux, uy, mask);
    rhoT: [P, FW] f32 tile (unit stride). Fills the 9 bf16 fstar planes and
    returns (fsv, mask_u8_plane).

    The channels are deinterleaved to unit-stride bf16 planes on the Scalar
    engine (f channels pre-scaled by -2/3) so DVE ops run in 2x/4x modes:
    tensor_tensor is 2x_1p in bf16 and tensor_scalar up to 4x; the
    unsupported-for-fast-modes scalar_tensor_tensor is avoided except for
    the f32 pressure term."""
    vec = nc.vector
    rv_f32 = _v3(rhoT[:])

    def t(name, dt=BF16):
        tl = scr.tile([P, FW], dt, tag=f"{tagp}{name}")
        return _v3(tl[:])

    # deinterleave (ACT): ff_i = -2/3 * f_i ; ux, uy, rho planes; mask cast
    ff = []
    for c in range(9):
        ffc = t(f"in{c}")
        nc.scalar.mul(ffc, fu12[:, :, c:c + 1], FCOEF)
        ff.append(ffc)
    ux = t("inux"); uy = t("inuy"); rvb = t("inrho")
    nc.scalar.copy(ux, fu12[:, :, 9:10])
    nc.scalar.copy(uy, fu12[:, :, 10:11])
    nc.scalar.copy(rvb, rv_f32)
    mkbt = scr.tile([P, FW], BF16, tag=f"{tagp}inmkb")
    nc.scalar.copy(_v3(mkbt[:]), fu12[:, :, 11:12])
    mku = scr.tile([P, FW], U8, tag=f"{tagp}mku")
    nc.scalar.copy(_v3(mku[:]), fu12[:, :, 11:12])

    r1 = t("r1"); r2 = t("r2"); t1 = t("t1"); t2 = t("t2")
    usqr = t("usqr"); Pv = t("P", FP32); sv = t("s"); dv = t("d")
    rsv = t("rs"); rdv = t("rd"); a5 = t("A5"); a6 = t("A6")
    pw9 = t("pw9"); pw36 = t("pw36"); pw49 = t("pw49", FP32)
    g1 = t("G1"); g2 = t("G2"); g5 = t("G5"); g6 = t("G6")
    aa1 = t("AA1"); aa2 = t("AA2"); aa5 = t("AA5"); aa6 = t("AA6")
    rr1 = t("RR1"); rr2 = t("RR2"); rr5 = t("RR5"); rr6 = t("RR6")
    feqv = []
    for i in range(1, 9):
        fq = t(f"feq{i}")
        feqv.append(fq)

    vec.tensor_tensor(r1, rvb, ux, AL.mult)
    vec.tensor_tensor(r2, rvb, uy, AL.mult)
    vec.tensor_tensor(t1, ux, r1, AL.mult)
    vec.tensor_tensor(t2, uy, r2, AL.mult)
    vec.tensor_tensor(usqr, t1, t2, AL.add)
    vec.scalar_tensor_tensor(Pv, usqr, -1.5, rv_f32, AL.mult, AL.add)
    vec.tensor_tensor(sv, ux, uy, AL.add)
    vec.tensor_tensor(dv, ux, uy, AL.subtract)
    vec.tensor_tensor(rsv, r1, r2, AL.add)
    vec.tensor_tensor(rdv, r1, r2, AL.subtract)
    vec.tensor_tensor(a5, sv, rsv, AL.mult)
    vec.tensor_tensor(a6, dv, rdv, AL.mult)
    nc.scalar.mul(pw9, Pv, W1P)
    nc.scalar.mul(pw36, Pv, W5P)
    nc.scalar.mul(pw49, Pv, W0P)
    vec.tensor_scalar_mul(aa1, t1, 4.5 * W1P)
    vec.tensor_scalar_mul(aa2, t2, 4.5 * W1P)
    vec.tensor_scalar_mul(aa5, a5, 4.5 * W5P)
    vec.tensor_scalar_mul(aa6, a6, 4.5 * W5P)
    vec.tensor_scalar_mul(rr1, r1, 3 * W1P)
    vec.tensor_scalar_mul(rr2, r2, 3 * W1P)
    vec.tensor_scalar_mul(rr5, rsv, 3 * W5P)
    vec.tensor_scalar_mul(rr6, rdv, 3 * W5P)
    vec.tensor_tensor(g1, aa1, pw9, AL.add)
    vec.tensor_tensor(g2, aa2, pw9, AL.add)
    vec.tensor_tensor(g5, aa5, pw36, AL.add)
    vec.tensor_tensor(g6, aa6, pw36, AL.add)
    # feq' (pre-scaled by 1/tau); index in feqv is dir-1
    vec.tensor_tensor(feqv[0], g1, rr1, AL.add)       # 1
    vec.tensor_tensor(feqv[2], g1, rr1, AL.subtract)  # 3
    vec.tensor_tensor(feqv[1], g2, rr2, AL.add)       # 2
    vec.tensor_tensor(feqv[3], g2, rr2, AL.subtract)  # 4
    vec.tensor_tensor(feqv[4], g5, rr5, AL.add)       # 5
    vec.tensor_tensor(feqv[6], g5, rr5, AL.subtract)  # 7
    vec.tensor_tensor(feqv[5], g6, rr6, AL.subtract)  # 6
    vec.tensor_tensor(feqv[7], g6, rr6, AL.add)       # 8
    fsv = [_v3(fs[:]) for fs in fstar]
    vec.tensor_tensor(fsv[0], ff[0], pw49, AL.add)
    for i in range(1, 9):
        vec.tensor_tensor(fsv[i], ff[i], feqv[i - 1], AL.add)
    return fsv, mku, mkbt


def _lift(nc, scr, P, OW, ov, tagp):
    """rho/u lift from the 9 selected slots of ov ([P, OW, 12] view)."""
    vec = nc.vector

    def t(name):
        tl = scr.tile([P, OW], FP32, tag=f"{tagp}{name}")
        return _v3(tl[:])

    # reuse collision scratch tags (those tiles are dead by lift time)
    av = t("feq1"); bv = t("feq2"); cv = t("feq3"); ddv = t("feq4")
    m1 = t("feq5"); m2 = t("feq6")
    t01 = t("feq7"); t23 = t("feq8"); t45 = t("G1"); t67 = t("G2")
    t03 = t("G5"); t47 = t("G6"); t07 = t("pw9"); inv = t("pw36")
    s_ = [ov[:, :, i:i + 1] for i in range(9)]
    vec.tensor_tensor(av, s_[1], s_[3], AL.subtract)
    vec.tensor_tensor(bv, s_[2], s_[4], AL.subtract)
    vec.tensor_tensor(cv, s_[5], s_[7], AL.subtract)
    vec.tensor_tensor(ddv, s_[8], s_[6], AL.subtract)
    vec.tensor_tensor(m1, av, cv, AL.add)
    vec.tensor_tensor(m1, m1, ddv, AL.add)
    vec.tensor_tensor(m2, bv, cv, AL.add)
    vec.tensor_tensor(m2, m2, ddv, AL.subtract)
    vec.tensor_tensor(t01, s_[0], s_[1], AL.add)
    vec.tensor_tensor(t23, s_[2], s_[3], AL.add)
    vec.tensor_tensor(t45, s_[4], s_[5], AL.add)
    vec.tensor_tensor(t67, s_[6], s_[7], AL.add)
    vec.tensor_tensor(t03, t01, t23, AL.add)
    vec.tensor_tensor(t47, t45, t67, AL.add)
    vec.tensor_tensor(t07, t03, t47, AL.add)
    vec.tensor_tensor(ov[:, :, 9:10], t07, s_[8], AL.add)
    vec.reciprocal_approx_fast(inv, ov[:, :, 9:10])
    vec.tensor_tensor(ov[:, :, 10:11], m1, inv, AL.mult)
    vec.tensor_tensor(ov[:, :, 11:12], m2, inv, AL.mult)


def _build_program():
    nc = bacc.Bacc(None)

    fu_d = nc.declare_dram_parameter("fu", [SLAB, YP, 12], BF16, isOutput=False)
    rho_d = nc.declare_dram_parameter("rho", [SLAB, YP], FP32, isOutput=False)
    # shm: [128, 256] two shifted identities: cols 0:128 -> S[m]=in[m-1]
    # (for ex=+1), cols 128:256 -> S[m]=in[m+1] (for ex=-1)
    shm_d = nc.declare_dram_parameter("shm", [128, SHM_COLS], BF16, isOutput=False)
    out_d = nc.declare_dram_parameter("out", [R, NY, 12], FP32, isOutput=True)

    with tile.TileContext(nc) as tc, tc.tile_pool(name="cst", bufs=1) as cst:
        shm = cst.tile([128, SHM_COLS], BF16)
        nc.sync.dma_start(out=shm[:], in_=shm_d[:, :])
        # ======================= main tiles =======================
        with (
            tc.tile_pool(name="io", bufs=2) as io,
            tc.tile_pool(name="pln", bufs=1) as pln,
            tc.tile_pool(name="psS", bufs=1, space="PSUM") as psS,
            tc.tile_pool(name="scr", bufs=1) as scr,
        ):
            it = 0
            for tb in TILE_BASES:
                for ch in range(NCHUNK):
                    c0 = ch * W
                    fuT = io.tile([128, F * 12], BF16, tag="fuT")
                    rhoT = io.tile([128, F], FP32, tag="rhoT")
                    outT = io.tile([128, W * 12], FP32, tag="outT")
                    nc.sync.dma_start(out=fuT[:], in_=fu_d[tb:tb + 128, c0:c0 + F, :].rearrange("r y c -> r (y c)"))
                    nc.sync.dma_start(out=rhoT[:], in_=rho_d[tb:tb + 128, c0:c0 + F])
                    fstar = [pln.tile([128, F], BF16, tag=f"fs{i}", name=f"fs{i}")
                             for i in range(9)]

                    fu12 = fuT[:].rearrange("p (x c) -> p x c", c=12)
                    fsv, mku, _mkb = _collide(nc, scr, 128, F, fu12, rhoT, fstar, "m_")

                    ov = outT[:].rearrange("p (x c) -> p x c", c=12)
                    mk = _v3(mku[:])[:, 1:1 + W, :]
                    for i in range(9):
                        exi, eyi = EX[i], EY[i]
                        ysl = slice(1 - eyi, 1 - eyi + W)
                        if exi == 0:
                            src = fsv[i][:, ysl, :]
                        else:
                            sp = psS.tile([128, W], FP32, tag=f"S{i}",
                                          name=f"S{i}")
                            wcol = slice(0, 128) if exi == 1 else slice(128, 256)
                            nc.tensor.matmul(sp[:], shm[:, wcol],
                                             fstar[i][:, ysl])
                            src = _v3(sp[:])
                        nc.scalar.copy(ov[:, :, i:i + 1], src)
                    for i in range(1, 9):
                        nc.vector.copy_predicated(ov[:, :, i:i + 1], mk,
                                                  fsv[OPP[i]][:, 1:1 + W, :])

                    _lift(nc, scr, 128, W, ov, "m_")

                    # valid out rows are partitions 1..126 (slab rows tb+1..)
                    st_eng = nc.sync
                    st_eng.dma_start(
                        out=out_d[tb:tb + 126, c0:c0 + W, :].rearrange(
                            "r y c -> r (y c)"),
                        in_=outT[1:127, :])
                    it += 1

            # ========== fix-up pass (out slab rows 127..130) ==========
            # shares pool tags with the main loop: behaves like a 9th
            # iteration, so its loads prefetch during the main loop and no
            # pool-transition barrier is created.
            PF = FX_PF
            PO = FX_PO
            fxfu = io.tile([PF, FX_F * 12], BF16, tag="fuT")
            fxrho = io.tile([PF, FX_F], FP32, tag="rhoT")
            fxout = io.tile([PO, FX_W * 12], FP32, tag="outT")
            # packed loads: partitions (seg, j) <- row FX_FS0+j, cols seg*FX_W
            for sg in range(FX_SEG):
                c0 = sg * FX_W
                nc.sync.dma_start(
                    out=fxfu[sg * FX_NFS:(sg + 1) * FX_NFS, :],
                    in_=fu_d[FX_FS0:FX_FS0 + FX_NFS, c0:c0 + FX_F, :].rearrange(
                        "r y c -> r (y c)"))
                nc.sync.dma_start(
                    out=fxrho[sg * FX_NFS:(sg + 1) * FX_NFS, :],
                    in_=rho_d[FX_FS0:FX_FS0 + FX_NFS, c0:c0 + FX_F])
            fxstar = [pln.tile([PF, FX_F], BF16, tag=f"fs{i}", name=f"fxs{i}")
                      for i in range(9)]

            fv12 = fxfu[:].rearrange("p (x c) -> p x c", c=12)
            _, _fxmku, fxmkb = _collide(nc, scr, PF, FX_F, fv12, fxrho, fxstar, "m_")

            # stream shift: out q = sg*4+jj is slab row 127+jj; source f_star
            # partition sg*6 + (jj+1-ex), free offset 1-ey
            ov = fxout[:].rearrange("p (x c) -> p x c", c=12)
            PBASE = {1: 256, 0: 256 + FX_PO, -1: 256 + 2 * FX_PO}
            for i in range(9):
                exi, eyi = EX[i], EY[i]
                ysl = slice(1 - eyi, 1 - eyi + FX_W)
                sp = psS.tile([PO, FX_W], FP32, tag="fxSp", name=f"fxS{i}")
                b = PBASE[exi]
                nc.tensor.matmul(sp[:], shm[0:PF, b:b + PO],
                                 fxstar[i][:, ysl])
                nc.scalar.copy(ov[:, :, i:i + 1], _v3(sp[:]))
            mkps = psS.tile([PO, FX_W], FP32, tag="fxBp", name="fxMk")
            nc.tensor.matmul(mkps[:], shm[0:PF, 256 + FX_PO:256 + 2 * FX_PO],
                             fxmkb[:, 1:1 + FX_W])
            fxmaskP = pln.tile([PO, FX_W], U8, tag="fxmaskP")
            nc.scalar.copy(_v3(fxmaskP[:]), _v3(mkps[:]))
            mk = _v3(fxmaskP[:])
            for i in range(1, 9):
                spb = psS.tile([PO, FX_W], FP32, tag="fxBp", name=f"fxB{i}")
                nc.tensor.matmul(spb[:], shm[0:PF, 256 + FX_PO:256 + 2 * FX_PO],
                                 fxstar[OPP[i]][:, 1:1 + FX_W])
                nc.vector.copy_predicated(ov[:, :, i:i + 1], mk, _v3(spb[:]))

            _lift(nc, scr, PO, FX_W, ov, "m_")

            # out slab rows 127..130 = out_d rows 126..129
            for sg in range(FX_SEG):
                st_eng = nc.sync
                st_eng.dma_start(
                    out=out_d[FX_OUT0 - 1:FX_OUT0 - 1 + FX_NOUT,
                              sg * FX_W:(sg + 1) * FX_W, :].rearrange(
                        "r y c -> r (y c)"),
                    in_=fxout[sg * FX_NOUT:(sg + 1) * FX_NOUT, :])

    nc.finalize()
    return nc


_NC_CACHE = None


def _get_nc():
    global _NC_CACHE
    if _NC_CACHE is None:
        _NC_CACHE = _build_program()
    return _NC_CACHE


def _shm_np():
    import ml_dtypes
    m = np.zeros((128, SHM_COLS), np.float32)
    for i in range(1, 128):
        m[i - 1, i] = 1.0          # S[m] = in[m-1]
    for i in range(0, 127):
        m[i + 1, 128 + i] = 1.0    # S[m] = in[m+1]
    # fix-up permutations: out q = sg*FX_NOUT+jj <- src k = sg*FX_NFS+jj+1-ex
    for bi, exi in enumerate((1, 0, -1)):
        base = 256 + FX_PO * bi
        for sg in range(FX_SEG):
            for jj in range(FX_NOUT):
                q = sg * FX_NOUT + jj
                k = sg * FX_NFS + jj + 1 - exi
                m[k, base + q] = 1.0
    return m.astype(ml_dtypes.bfloat16)


def _pad_slab(arr, lo, hi):
    """rows [lo-1, hi+1) with x wraparound, then 1-col y wraparound halo."""
    rows = np.take(arr, np.arange(lo - 1, hi + 1), axis=0, mode="wrap")
    return np.concatenate([rows[:, -1:], rows, rows[:, :1]], axis=1)


def kernel(f, rho, u, obstacle_mask, _trace=False):
    import ml_dtypes
    f = np.asarray(f, dtype=np.float32)
    rho = np.asarray(rho, dtype=np.float32)
    u = np.asarray(u, dtype=np.float32)
    maskf = np.asarray(obstacle_mask).astype(np.float32)
    fu = np.concatenate([f, u, maskf[..., None]],
                        axis=-1).astype(ml_dtypes.bfloat16)  # [NX, NY, 12]

    shm = _shm_np()
    in_maps = []
    for k in range(NCORES):
        lo, hi = k * R, (k + 1) * R
        in_maps.append({
            "fu": np.ascontiguousarray(_pad_slab(fu, lo, hi)),
            "rho": np.ascontiguousarray(_pad_slab(rho, lo, hi)),
            "shm": shm,
        })

    nc = _get_nc()
    res = run_bass_kernel_spmd(nc, in_maps, list(range(NCORES)),
                               trace=bool(_trace))
    out = np.concatenate([res.results[k]["out"] for k in range(NCORES)], axis=0)
    if _trace:
        return out, res
    return out


# revision 21
# speedup vs baseline: 274.8978x; 1.0664x over previous
"""D2Q9 Lattice-Boltzmann single step (collide + stream + bounce-back + lift)
on 8 Trainium2 NeuronCores.

Contract: kernel(**inputs) takes FULL inputs
  f [2048,2048,9] f32, rho [2048,2048] f32, u [2048,2048,2] f32,
  obstacle_mask [2048,2048] bool
and returns the FULL output [2048,2048,12] f32
  (f_new[9], rho_new, u_new[2] packed on the last axis).

Sharding: 1-D domain decomposition over the first spatial axis; each core
gets 256 rows plus a 1-row halo on each side (built host-side with
wraparound, so the device kernel is a purely local stencil). The 1-column
y-halo is also host-padded. Host packs f, rho, u into one 12-channel
tensor so each tile needs just two loads (data + mask) and one store.

Streaming's +-1 row shift is done with a shifted-identity matmul on the
(otherwise idle) TensorEngine into PSUM; the +-1 column shift is a free-dim
AP offset. Bounce-back is copy_predicated on the obstacle mask."""

import numpy as np
import concourse.bass as bass
import concourse.bacc as bacc
import concourse.mybir as mybir
from concourse import tile
from concourse.bass_utils import run_bass_kernel_spmd

NX = 2048
NY = 2048
NCORES = 8
R = NX // NCORES          # 256 rows per core
SLAB = R + 2              # 258 rows incl halos
YP = NY + 2               # 2050 cols incl halos

TAU = 0.6
INV_TAU = 1.0 / TAU       # 5/3
FCOEF = 1.0 - INV_TAU     # -2/3   (f coefficient in f* = FCOEF*f + INV_TAU*feq)
W1P = INV_TAU * (1.0 / 9.0)    # 5/27  (axis dirs, scaled by 1/tau)
W5P = INV_TAU * (1.0 / 36.0)   # 5/108 (diag dirs)
W0P = INV_TAU * (4.0 / 9.0)    # 20/27 (rest dir)

EX = [0, 1, 0, -1, 0, 1, -1, -1, 1]
EY = [0, 0, 1, 0, -1, 1, 1, -1, -1]
OPP = [0, 3, 4, 1, 2, 7, 8, 5, 6]

W = 512                   # y-chunk width
NCHUNK = NY // W          # 4
F = W + 2                 # chunk width incl y-halo
FP32 = mybir.dt.float32
BF16 = mybir.dt.bfloat16
U8 = mybir.dt.uint8

# main x-tiles: f_star row ranges [base, base+128); fix-up covers the gap
TILE_BASES = [0, 130]
# fix-up: f_star slab rows 126..131 (6 rows), out slab rows 127..130
FX_FS0 = 126
FX_NFS = 6
FX_OUT0 = 127
FX_NOUT = 4
FX_SEG = 8
FX_W = NY // FX_SEG
FX_F = FX_W + 2
FX_PO = FX_SEG * FX_NOUT
FX_PF = FX_SEG * FX_NFS
SHM_COLS = 256 + 3 * FX_PO

AL = mybir.AluOpType


def _v3(ap):
    """view a [P, N] AP as [P, N, 1] so all operands rank-match."""
    return ap.rearrange("p (x c) -> p x c", c=1)


def _collide(nc, scr, P, FW, fu12, rhoT, fstar, tagp):
    """Collision stage. fu12: [P, FW, 12] bf16 view (f0..f8, ux, uy, mask);
    rhoT: [P, FW] f32 tile (unit stride). Fills the 9 bf16 fstar planes and
    returns (fsv, mask_u8_plane).

    The channels are deinterleaved to unit-stride bf16 planes on the Scalar
    engine (f channels pre-scaled by -2/3) so DVE ops run in 2x/4x modes:
    tensor_tensor is 2x_1p in bf16 and tensor_scalar up to 4x; the
    unsupported-for-fast-modes scalar_tensor_tensor is avoided except for
    the f32 pressure term."""
    vec = nc.vector
    rv_f32 = _v3(rhoT[:])

    def t(name, dt=BF16):
        tl = scr.tile([P, FW], dt, tag=f"{tagp}{name}")
        return _v3(tl[:])

    # deinterleave (ACT): ff_i = -2/3 * f_i ; ux, uy, rho planes; mask cast
    ff = []
    for c in range(9):
        ffc = t(f"in{c}")
        nc.scalar.mul(ffc, fu12[:, :, c:c + 1], FCOEF)
        ff.append(ffc)
    ux = t("inux"); uy = t("inuy"); rvb = t("inrho")
    nc.scalar.copy(ux, fu12[:, :, 9:10])
    nc.scalar.copy(uy, fu12[:, :, 10:11])
    nc.scalar.copy(rvb, rv_f32)
    mkbt = scr.tile([P, FW], BF16, tag=f"{tagp}inmkb")
    nc.scalar.copy(_v3(mkbt[:]), fu12[:, :, 11:12])
    mku = scr.tile([P, FW], U8, tag=f"{tagp}mku")
    nc.scalar.copy(_v3(mku[:]), fu12[:, :, 11:12])

    r1 = t("r1"); r2 = t("r2"); t1 = t("t1"); t2 = t("t2")
    usqr = t("usqr"); Pv = t("P", FP32); sv = t("s"); dv = t("d")
    rsv = t("rs"); rdv = t("rd"); a5 = t("A5"); a6 = t("A6")
    pw9 = t("pw9"); pw36 = t("pw36"); pw49 = t("pw49", FP32)
    g1 = t("G1"); g2 = t("G2"); g5 = t("G5"); g6 = t("G6")
    aa1 = t("AA1"); aa2 = t("AA2"); aa5 = t("AA5"); aa6 = t("AA6")
    rr1 = t("RR1"); rr2 = t("RR2"); rr5 = t("RR5"); rr6 = t("RR6")
    feqv = []
    for i in range(1, 9):
        fq = t(f"feq{i}")
        feqv.append(fq)

    vec.tensor_tensor(r1, rvb, ux, AL.mult)
    vec.tensor_tensor(r2, rvb, uy, AL.mult)
    vec.tensor_tensor(t1, ux, r1, AL.mult)
    vec.tensor_tensor(t2, uy, r2, AL.mult)
    vec.tensor_tensor(usqr, t1, t2, AL.add)
    vec.scalar_tensor_tensor(Pv, usqr, -1.5, rv_f32, AL.mult, AL.add)
    vec.tensor_tensor(sv, ux, uy, AL.add)
    vec.tensor_tensor(dv, ux, uy, AL.subtract)
    vec.tensor_tensor(rsv, r1, r2, AL.add)
    vec.tensor_tensor(rdv, r1, r2, AL.subtract)
    vec.tensor_tensor(a5, sv, rsv, AL.mult)
    vec.tensor_tensor(a6, dv, rdv, AL.mult)
    nc.scalar.mul(pw9, Pv, W1P)
    nc.scalar.mul(pw36, Pv, W5P)
    nc.scalar.mul(pw49, Pv, W0P)
    vec.tensor_scalar_mul(aa1, t1, 4.5 * W1P)
    vec.tensor_scalar_mul(aa2, t2, 4.5 * W1P)
    vec.tensor_scalar_mul(aa5, a5, 4.5 * W5P)
    vec.tensor_scalar_mul(aa6, a6, 4.5 * W5P)
    vec.tensor_scalar_mul(rr1, r1, 3 * W1P)
    vec.tensor_scalar_mul(rr2, r2, 3 * W1P)
    vec.tensor_scalar_mul(rr5, rsv, 3 * W5P)
    vec.tensor_scalar_mul(rr6, rdv, 3 * W5P)
    vec.tensor_tensor(g1, aa1, pw9, AL.add)
    vec.tensor_tensor(g2, aa2, pw9, AL.add)
    vec.tensor_tensor(g5, aa5, pw36, AL.add)
    vec.tensor_tensor(g6, aa6, pw36, AL.add)
    # feq' (pre-scaled by 1/tau); index in feqv is dir-1
    vec.tensor_tensor(feqv[0], g1, rr1, AL.add)       # 1
    vec.tensor_tensor(feqv[2], g1, rr1, AL.subtract)  # 3
    vec.tensor_tensor(feqv[1], g2, rr2, AL.add)       # 2
    vec.tensor_tensor(feqv[3], g2, rr2, AL.subtract)  # 4
    vec.tensor_tensor(feqv[4], g5, rr5, AL.add)       # 5
    vec.tensor_tensor(feqv[6], g5, rr5, AL.subtract)  # 7
    vec.tensor_tensor(feqv[5], g6, rr6, AL.subtract)  # 6
    vec.tensor_tensor(feqv[7], g6, rr6, AL.add)       # 8
    fsv = [_v3(fs[:]) for fs in fstar]
    vec.tensor_tensor(fsv[0], ff[0], pw49, AL.add)
    for i in range(1, 9):
        vec.tensor_tensor(fsv[i], ff[i], feqv[i - 1], AL.add)
    return fsv, mku, mkbt


def _lift(nc, scr, P, OW, ov, tagp):
    """rho/u lift from the 9 selected slots of ov ([P, OW, 12] view)."""
    vec = nc.vector

    def t(name):
        tl = scr.tile([P, OW], FP32, tag=f"{tagp}{name}")
        return _v3(tl[:])

    # reuse collision scratch tags (those tiles are dead by lift time)
    av = t("feq1"); bv = t("feq2"); cv = t("feq3"); ddv = t("feq4")
    m1 = t("feq5"); m2 = t("feq6")
    t01 = t("feq7"); t23 = t("feq8"); t45 = t("G1"); t67 = t("G2")
    t03 = t("G5"); t47 = t("G6"); t07 = t("pw9"); inv = t("pw36")
    rhoF = t("lrho")
    s_ = [ov[:, :, i:i + 1] for i in range(9)]
    vec.tensor_tensor(av, s_[1], s_[3], AL.subtract)
    vec.tensor_tensor(bv, s_[2], s_[4], AL.subtract)
    vec.tensor_tensor(cv, s_[5], s_[7], AL.subtract)
    vec.tensor_tensor(ddv, s_[8], s_[6], AL.subtract)
    vec.tensor_tensor(m1, av, cv, AL.add)
    vec.tensor_tensor(m1, m1, ddv, AL.add)
    vec.tensor_tensor(m2, bv, cv, AL.add)
    vec.tensor_tensor(m2, m2, ddv, AL.subtract)
    vec.tensor_tensor(t01, s_[0], s_[1], AL.add)
    vec.tensor_tensor(t23, s_[2], s_[3], AL.add)
    vec.tensor_tensor(t45, s_[4], s_[5], AL.add)
    vec.tensor_tensor(t67, s_[6], s_[7], AL.add)
    vec.tensor_tensor(t03, t01, t23, AL.add)
    vec.tensor_tensor(t47, t45, t67, AL.add)
    vec.tensor_tensor(t07, t03, t47, AL.add)
    vec.tensor_tensor(rhoF, t07, s_[8], AL.add)
    nc.scalar.copy(ov[:, :, 9:10], rhoF)
    vec.reciprocal_approx_fast(inv, rhoF)
    vec.tensor_tensor(ov[:, :, 10:11], m1, inv, AL.mult)
    vec.tensor_tensor(ov[:, :, 11:12], m2, inv, AL.mult)


def _build_program():
    nc = bacc.Bacc(None)

    fu_d = nc.declare_dram_parameter("fu", [SLAB, YP, 12], BF16, isOutput=False)
    rho_d = nc.declare_dram_parameter("rho", [SLAB, YP], FP32, isOutput=False)
    # shm: [128, 256] two shifted identities: cols 0:128 -> S[m]=in[m-1]
    # (for ex=+1), cols 128:256 -> S[m]=in[m+1] (for ex=-1)
    shm_d = nc.declare_dram_parameter("shm", [128, SHM_COLS], BF16, isOutput=False)
    out_d = nc.declare_dram_parameter("out", [R, NY, 12], BF16, isOutput=True)

    with tile.TileContext(nc) as tc, tc.tile_pool(name="cst", bufs=1) as cst:
        shm = cst.tile([128, SHM_COLS], BF16)
        nc.sync.dma_start(out=shm[:], in_=shm_d[:, :])
        # ======================= main tiles =======================
        with (
            tc.tile_pool(name="io", bufs=2) as io,
            tc.tile_pool(name="pln", bufs=1) as pln,
            tc.tile_pool(name="psS", bufs=1, space="PSUM") as psS,
            tc.tile_pool(name="scr", bufs=1) as scr,
        ):
            it = 0
            for tb in TILE_BASES:
                for ch in range(NCHUNK):
                    c0 = ch * W
                    fuT = io.tile([128, F * 12], BF16, tag="fuT")
                    rhoT = io.tile([128, F], FP32, tag="rhoT")
                    outT = io.tile([128, W * 12], BF16, tag="outT")
                    nc.sync.dma_start(out=fuT[:], in_=fu_d[tb:tb + 128, c0:c0 + F, :].rearrange("r y c -> r (y c)"))
                    nc.sync.dma_start(out=rhoT[:], in_=rho_d[tb:tb + 128, c0:c0 + F])
                    fstar = [pln.tile([128, F], BF16, tag=f"fs{i}", name=f"fs{i}")
                             for i in range(9)]

                    fu12 = fuT[:].rearrange("p (x c) -> p x c", c=12)
                    fsv, mku, _mkb = _collide(nc, scr, 128, F, fu12, rhoT, fstar, "m_")

                    ov = outT[:].rearrange("p (x c) -> p x c", c=12)
                    mk = _v3(mku[:])[:, 1:1 + W, :]
                    for i in range(9):
                        exi, eyi = EX[i], EY[i]
                        ysl = slice(1 - eyi, 1 - eyi + W)
                        if exi == 0:
                            src = fsv[i][:, ysl, :]
                        else:
                            sp = psS.tile([128, W], FP32, tag=f"S{i}",
                                          name=f"S{i}")
                            wcol = slice(0, 128) if exi == 1 else slice(128, 256)
                            nc.tensor.matmul(sp[:], shm[:, wcol],
                                             fstar[i][:, ysl])
                            src = _v3(sp[:])
                        nc.scalar.copy(ov[:, :, i:i + 1], src)
                    for i in range(1, 9):
                        nc.vector.copy_predicated(ov[:, :, i:i + 1], mk,
                                                  fsv[OPP[i]][:, 1:1 + W, :])

                    _lift(nc, scr, 128, W, ov, "m_")

                    # valid out rows are partitions 1..126 (slab rows tb+1..)
                    st_eng = nc.sync
                    st_eng.dma_start(
                        out=out_d[tb:tb + 126, c0:c0 + W, :].rearrange(
                            "r y c -> r (y c)"),
                        in_=outT[1:127, :])
                    it += 1

            # ========== fix-up pass (out slab rows 127..130) ==========
            # shares pool tags with the main loop: behaves like a 9th
            # iteration, so its loads prefetch during the main loop and no
            # pool-transition barrier is created.
            PF = FX_PF
            PO = FX_PO
            fxfu = io.tile([PF, FX_F * 12], BF16, tag="fuT")
            fxrho = io.tile([PF, FX_F], FP32, tag="rhoT")
            fxout = io.tile([PO, FX_W * 12], BF16, tag="outT")
            # packed loads: partitions (seg, j) <- row FX_FS0+j, cols seg*FX_W
            for sg in range(FX_SEG):
                c0 = sg * FX_W
                nc.sync.dma_start(
                    out=fxfu[sg * FX_NFS:(sg + 1) * FX_NFS, :],
                    in_=fu_d[FX_FS0:FX_FS0 + FX_NFS, c0:c0 + FX_F, :].rearrange(
                        "r y c -> r (y c)"))
                nc.sync.dma_start(
                    out=fxrho[sg * FX_NFS:(sg + 1) * FX_NFS, :],
                    in_=rho_d[FX_FS0:FX_FS0 + FX_NFS, c0:c0 + FX_F])
            fxstar = [pln.tile([PF, FX_F], BF16, tag=f"fs{i}", name=f"fxs{i}")
                      for i in range(9)]

            fv12 = fxfu[:].rearrange("p (x c) -> p x c", c=12)
            _, _fxmku, fxmkb = _collide(nc, scr, PF, FX_F, fv12, fxrho, fxstar, "m_")

            # stream shift: out q = sg*4+jj is slab row 127+jj; source f_star
            # partition sg*6 + (jj+1-ex), free offset 1-ey
            ov = fxout[:].rearrange("p (x c) -> p x c", c=12)
            PBASE = {1: 256, 0: 256 + FX_PO, -1: 256 + 2 * FX_PO}
            for i in range(9):
                exi, eyi = EX[i], EY[i]
                ysl = slice(1 - eyi, 1 - eyi + FX_W)
                sp = psS.tile([PO, FX_W], FP32, tag="fxSp", name=f"fxS{i}")
                b = PBASE[exi]
                nc.tensor.matmul(sp[:], shm[0:PF, b:b + PO],
                                 fxstar[i][:, ysl])
                nc.scalar.copy(ov[:, :, i:i + 1], _v3(sp[:]))
            mkps = psS.tile([PO, FX_W], FP32, tag="fxBp", name="fxMk")
            nc.tensor.matmul(mkps[:], shm[0:PF, 256 + FX_PO:256 + 2 * FX_PO],
                             fxmkb[:, 1:1 + FX_W])
            fxmaskP = pln.tile([PO, FX_W], U8, tag="fxmaskP")
            nc.scalar.copy(_v3(fxmaskP[:]), _v3(mkps[:]))
            mk = _v3(fxmaskP[:])
            for i in range(1, 9):
                spb = psS.tile([PO, FX_W], FP32, tag="fxBp", name=f"fxB{i}")
                nc.tensor.matmul(spb[:], shm[0:PF, 256 + FX_PO:256 + 2 * FX_PO],
                                 fxstar[OPP[i]][:, 1:1 + FX_W])
                nc.vector.copy_predicated(ov[:, :, i:i + 1], mk, _v3(spb[:]))

            _lift(nc, scr, PO, FX_W, ov, "m_")

            # out slab rows 127..130 = out_d rows 126..129
            for sg in range(FX_SEG):
                st_eng = nc.sync
                st_eng.dma_start(
                    out=out_d[FX_OUT0 - 1:FX_OUT0 - 1 + FX_NOUT,
                              sg * FX_W:(sg + 1) * FX_W, :].rearrange(
                        "r y c -> r (y c)"),
                    in_=fxout[sg * FX_NOUT:(sg + 1) * FX_NOUT, :])

    nc.finalize()
    return nc


_NC_CACHE = None


def _get_nc():
    global _NC_CACHE
    if _NC_CACHE is None:
        _NC_CACHE = _build_program()
    return _NC_CACHE


def _shm_np():
    import ml_dtypes
    m = np.zeros((128, SHM_COLS), np.float32)
    for i in range(1, 128):
        m[i - 1, i] = 1.0          # S[m] = in[m-1]
    for i in range(0, 127):
        m[i + 1, 128 + i] = 1.0    # S[m] = in[m+1]
    # fix-up permutations: out q = sg*FX_NOUT+jj <- src k = sg*FX_NFS+jj+1-ex
    for bi, exi in enumerate((1, 0, -1)):
        base = 256 + FX_PO * bi
        for sg in range(FX_SEG):
            for jj in range(FX_NOUT):
                q = sg * FX_NOUT + jj
                k = sg * FX_NFS + jj + 1 - exi
                m[k, base + q] = 1.0
    return m.astype(ml_dtypes.bfloat16)


def _pad_slab(arr, lo, hi):
    """rows [lo-1, hi+1) with x wraparound, then 1-col y wraparound halo."""
    rows = np.take(arr, np.arange(lo - 1, hi + 1), axis=0, mode="wrap")
    return np.concatenate([rows[:, -1:], rows, rows[:, :1]], axis=1)


def kernel(f, rho, u, obstacle_mask, _trace=False):
    import ml_dtypes
    f = np.asarray(f, dtype=np.float32)
    rho = np.asarray(rho, dtype=np.float32)
    u = np.asarray(u, dtype=np.float32)
    maskf = np.asarray(obstacle_mask).astype(np.float32)
    fu = np.concatenate([f, u, maskf[..., None]],
                        axis=-1).astype(ml_dtypes.bfloat16)  # [NX, NY, 12]

    shm = _shm_np()
    in_maps = []
    for k in range(NCORES):
        lo, hi = k * R, (k + 1) * R
        in_maps.append({
            "fu": np.ascontiguousarray(_pad_slab(fu, lo, hi)),
            "rho": np.ascontiguousarray(_pad_slab(rho, lo, hi)),
            "shm": shm,
        })

    nc = _get_nc()
    res = run_bass_kernel_spmd(nc, in_maps, list(range(NCORES)),
                               trace=bool(_trace))
    out = np.concatenate([res.results[k]["out"] for k in range(NCORES)],
                     axis=0).astype(np.float32)
    if _trace:
        return out, res
    return out
